# revision 1
# baseline (speedup 1.0000x reference)
"""Trainium2 Bass kernel for Hash1d: out = x @ hashProj.

hashProj is an extremely sparse hash-projection matrix (one +-1 per row), so
out[b, e] = sum_{j: h(j)=e} sign_j * x[b, j] -- a signed segment-sum of x's
columns into E buckets.

Strategy (8 NeuronCores):
  * Host: extract the nonzero entries (col j, bucket e, value v) from
    hashProj, sort them by bucket, and shard *buckets* across the 8 cores
    (core i owns buckets [128*i, 128*(i+1))).  Output shards are disjoint,
    so no collective is needed.
  * Host hands core i a contiguous, transposed slab xs = x.T[cols of core i]
    (features on partitions) padded to a common chunk multiple, plus a tiny
    per-chunk "signed one-hot" matrix w [128 feats x 128 local buckets].
  * Device: for each 128-feature chunk, one 2 MiB contiguous DMA brings in
    xs[k] = [128, 4096]; the PE computes acc[, bank] += w[k].T @ xs[k] for
    the 8 PSUM banks (N=512 fp32 moving limit).  All chunks accumulate into
    one full-PSUM [128, 4096] tile, which is copied to SBUF and DMA'd out.
  * Everything is exact fp32 (products are x * +-1), so the result matches
    the fp32 reference to reordering error (~1e-7).

Device traffic per core: ~34 MiB in + 2 MiB out -> ~100 us at ~360 GB/s HBM,
which is at the memory roofline (hashProj's 64 MiB dense zeros never touch
the device).
"""

import numpy as np

BATCH = 4096
INPUT_DIM = 16384
EMB_SIZE = 1024
N_CORES = 8
BPC = EMB_SIZE // N_CORES  # buckets (output partitions) per core = 128
P = 128                    # features per chunk (PE contraction dim)
NFREE = 512                # fp32 moving-operand max free dim = one PSUM bank
NBANK = BATCH // NFREE     # 8 PSUM banks cover the batch

_prog_cache = {}


def _build_program(n_chunks):
    import concourse.bass as bass
    import concourse.tile as tile
    from concourse import bacc, mybir

    f32 = mybir.dt.float32
    nc = bacc.Bacc("TRN2", target_bir_lowering=False, debug=False)

    xs_d = nc.dram_tensor("xs", [n_chunks, P, BATCH], f32, kind="ExternalInput")
    w_d = nc.dram_tensor("w", [n_chunks, P, BPC], f32, kind="ExternalInput")
    out_d = nc.dram_tensor("out", [BPC, BATCH], f32, kind="ExternalOutput")

    with tile.TileContext(nc) as tc:
        with (
            tc.tile_pool(name="xpool", bufs=4) as xpool,
            tc.tile_pool(name="wpool", bufs=4) as wpool,
            tc.tile_pool(name="psum", bufs=1, space=bass.MemorySpace.PSUM) as ppool,
            tc.tile_pool(name="opool", bufs=1) as opool,
        ):
            acc = ppool.tile([BPC, BATCH], f32)
            for k in range(n_chunks):
                wt = wpool.tile([P, BPC], f32)
                nc.sync.dma_start(wt[:], w_d[k])
                xt = xpool.tile([P, BATCH], f32)
                nc.sync.dma_start(xt[:], xs_d[k])
                for n in range(NBANK):
                    nc.tensor.matmul(
                        acc[:, bass.ts(n, NFREE)],
                        wt[:],
                        xt[:, bass.ts(n, NFREE)],
                        start=(k == 0),
                        stop=(k == n_chunks - 1),
                    )
            out_t = opool.tile([BPC, BATCH], f32)
            for n in range(NBANK):
                nc.vector.tensor_copy(out_t[:, bass.ts(n, NFREE)], acc[:, bass.ts(n, NFREE)])
            nc.sync.dma_start(out_d[:], out_t[:])

    nc.compile()
    return nc


def _host_prep(x, hashProj):
    """Extract sparse entries, shard by bucket across cores, build per-core inputs."""
    x = np.ascontiguousarray(x, dtype=np.float32)
    hashProj = np.asarray(hashProj, dtype=np.float32)

    # General sparse decomposition: out = sum over nonzeros (j, e, v) of v * x[:, j].
    rows, cols = np.nonzero(hashProj)
    vals = hashProj[rows, cols].astype(np.float32)
    order = np.argsort(cols, kind="stable")
    rows, cols, vals = rows[order], cols[order], vals[order]

    core_of = cols // BPC
    counts = np.bincount(core_of, minlength=N_CORES)
    n_chunks = max(1, -(-int(counts.max()) // P))
    Lp = n_chunks * P

    xT = np.ascontiguousarray(x.T)  # [D, B]: feature-major for partition-dim DMA
    offs = np.zeros(N_CORES + 1, np.int64)
    np.cumsum(counts, out=offs[1:])

    in_maps = []
    for i in range(N_CORES):
        r = rows[offs[i]:offs[i + 1]]
        c = cols[offs[i]:offs[i + 1]]
        v = vals[offs[i]:offs[i + 1]]
        li = len(r)
        xs = np.zeros((Lp, BATCH), np.float32)
        w = np.zeros((Lp, BPC), np.float32)
        if li:
            xs[:li] = xT[r]
            w[np.arange(li), c - i * BPC] = v
        in_maps.append({
            "xs": xs.reshape(n_chunks, P, BATCH),
            "w": w.reshape(n_chunks, P, BPC),
        })
    return in_maps, n_chunks


def _run(x, hashProj, trace=False):
    from concourse.bass_utils import run_bass_kernel_spmd

    in_maps, n_chunks = _host_prep(x, hashProj)
    if n_chunks not in _prog_cache:
        _prog_cache[n_chunks] = _build_program(n_chunks)
    nc = _prog_cache[n_chunks]

    res = run_bass_kernel_spmd(nc, in_maps, list(range(N_CORES)), trace=trace)
    out_T = np.concatenate([res.results[i]["out"] for i in range(N_CORES)], axis=0)
    out = np.ascontiguousarray(out_T.T, dtype=np.float32)
    return out, res


def kernel(x, hashProj):
    out, _ = _run(x, hashProj)
    return out


# revision 9
# speedup vs baseline: 258.8823x; 258.8823x over previous
"""Trainium2 Bass kernel for Hash1d: out = x @ hashProj.

hashProj is an extremely sparse hash-projection matrix (one +-1 per row), so
out[b, e] = sum_{j: h(j)=e} sign_j * x[b, j] -- a signed segment-sum of x's
columns into E buckets.

Strategy (8 NeuronCores):
  * Host: extract the nonzero entries (col j, bucket e, value v) from
    hashProj, sort them by bucket, and shard *buckets* across the 8 cores
    (core i owns buckets [128*i, 128*(i+1))).  Output shards are disjoint,
    so no collective is needed.
  * Host hands core i a contiguous, transposed slab xs = x.T[cols of core i]
    (features on partitions) padded to a common chunk multiple, plus a tiny
    packed "signed one-hot" matrix w [128 feats x n_chunks*128 local buckets].
  * Device: xs is packed so each DMA group of G chunks is one contiguous-per-
    partition transfer; the PE computes acc[:, bank] += w_k.T @ xs_k for the
    8 PSUM banks (N=512 fp32 moving limit).  All chunks accumulate into one
    full-PSUM [128, 4096] tile, which is copied to SBUF and DMA'd out.
  * Everything is exact fp32 (products are x * +-1), so the result matches
    the fp32 reference to reordering error (~1e-7).

Device traffic per core: ~35 MiB in + 2 MiB out -> ~100 us at ~360 GB/s HBM,
which is at the memory roofline (hashProj's 64 MiB dense zeros never touch
the device).
"""

import numpy as np

BATCH = 4096
INPUT_DIM = 16384
EMB_SIZE = 1024
N_CORES = 8
BPC = EMB_SIZE // N_CORES  # buckets (output partitions) per core = 128
P = 128                    # features per chunk (PE contraction dim)
NFREE = 512                # fp32 moving-operand max free dim = one PSUM bank
NBANK = BATCH // NFREE     # 8 PSUM banks cover the batch
GROUP = 2                  # chunks per xs DMA (4 MiB transfers, best measured)
XBUFS = 4                  # xs group tiles in flight
W_ON_ACT = True            # issue w/out DMAs on the ACT HWDGE queue
XS_QUEUES = 1              # 1: all xs DMAs on sync; 2: alternate sync/scalar

_prog_cache = {}


def _chunk_groups(n_chunks):
    """Split chunk indices into DMA groups of size <= GROUP."""
    groups = []
    c = 0
    while c < n_chunks:
        g = min(GROUP, n_chunks - c)
        groups.append((c, g))
        c += g
    return groups


def _build_program(n_chunks, reps=1):
    import concourse.bass as bass
    import concourse.tile as tile
    from concourse import bacc, mybir

    f32 = mybir.dt.float32
    nc = bacc.Bacc("TRN2", target_bir_lowering=False, debug=False)

    # xs packed per group: [128 partitions, g*BATCH] contiguous per partition
    xs_d = nc.dram_tensor("xs", [n_chunks * P * BATCH], f32, kind="ExternalInput")
    # w packed: [128 feat partitions, n_chunks * BPC]
    w_d = nc.dram_tensor("w", [P, n_chunks * BPC], f32, kind="ExternalInput")
    out_d = nc.dram_tensor("out", [BPC, BATCH], f32, kind="ExternalOutput")

    groups = _chunk_groups(n_chunks)

    with tile.TileContext(nc) as tc:
        W_ENG = nc.scalar if W_ON_ACT else nc.sync
        with (
            tc.tile_pool(name="xpool", bufs=XBUFS) as xpool,
            tc.tile_pool(name="wpool", bufs=1) as wpool,
            tc.tile_pool(name="psum", bufs=1, space=bass.MemorySpace.PSUM) as ppool,
            tc.tile_pool(name="opool", bufs=1) as opool,
        ):
            def body(_i):
                wt = wpool.tile([P, n_chunks * BPC], f32)
                W_ENG.dma_start(wt[:], w_d[:])
                acc = ppool.tile([BPC, BATCH], f32)
                for gi, (c0, g) in enumerate(groups):
                    xt = xpool.tile([P, GROUP * BATCH], f32, tag="xs")
                    src = xs_d.ap()[c0 * P * BATCH:(c0 + g) * P * BATCH]
                    xeng = nc.scalar if (XS_QUEUES == 2 and gi % 2) else nc.sync
                    xeng.dma_start(
                        xt[:, :g * BATCH],
                        src.rearrange("(p n) -> p n", p=P),
                    )
                    for cl in range(g):
                        k = c0 + cl
                        for n in range(NBANK):
                            nc.tensor.matmul(
                                acc[:, bass.ts(n, NFREE)],
                                wt[:, bass.ts(k, BPC)],
                                xt[:, cl * BATCH + n * NFREE:cl * BATCH + (n + 1) * NFREE],
                                start=(k == 0),
                                stop=(k == n_chunks - 1),
                            )
                out_t = opool.tile([BPC, BATCH], f32)
                for n in range(NBANK):
                    nc.vector.tensor_copy(
                        out_t[:, bass.ts(n, NFREE)], acc[:, bass.ts(n, NFREE)]
                    )
                W_ENG.dma_start(out_d[:], out_t[:])

            if reps == 1:
                body(None)
            else:
                with tc.For_i(0, reps, 1) as i:
                    body(i)

    nc.compile()
    return nc


def _host_prep(x, hashProj):
    """Extract sparse entries, shard by bucket across cores, build per-core inputs."""
    x = np.ascontiguousarray(x, dtype=np.float32)
    hashProj = np.asarray(hashProj, dtype=np.float32)

    # General sparse decomposition: out = sum over nonzeros (j, e, v) of v * x[:, j].
    rows, cols = np.nonzero(hashProj)
    vals = hashProj[rows, cols].astype(np.float32)
    order = np.argsort(cols, kind="stable")
    rows, cols, vals = rows[order], cols[order], vals[order]

    core_of = cols // BPC
    counts = np.bincount(core_of, minlength=N_CORES)
    n_chunks = max(1, -(-int(counts.max()) // P))
    Lp = n_chunks * P

    xT = np.ascontiguousarray(x.T)  # [D, B]: feature-major for partition-dim DMA
    offs = np.zeros(N_CORES + 1, np.int64)
    np.cumsum(counts, out=offs[1:])

    groups = _chunk_groups(n_chunks)

    in_maps = []
    for i in range(N_CORES):
        r = rows[offs[i]:offs[i + 1]]
        c = cols[offs[i]:offs[i + 1]]
        v = vals[offs[i]:offs[i + 1]]
        li = len(r)
        # chunk-major staging: row (k*P + p) = feature p of chunk k
        xs_rows = np.zeros((Lp, BATCH), np.float32)
        if li:
            xs_rows[:li] = xT[r]
        # pack per group: [p, c_local, n] so each group is contiguous per partition
        xs = np.empty(Lp * BATCH, np.float32)
        pos = 0
        for c0, g in groups:
            blk = xs_rows[c0 * P:(c0 + g) * P].reshape(g, P, BATCH)
            xs[pos:pos + g * P * BATCH] = (
                blk.transpose(1, 0, 2).reshape(-1)
            )
            pos += g * P * BATCH
        w = np.zeros((Lp, BPC), np.float32)
        if li:
            w[np.arange(li), c - i * BPC] = v
        # pack w: [p, k*BPC + m]
        w2 = np.ascontiguousarray(
            w.reshape(n_chunks, P, BPC).transpose(1, 0, 2).reshape(P, n_chunks * BPC)
        )
        in_maps.append({"xs": xs, "w": w2})
    return in_maps, n_chunks


def _run(x, hashProj, trace=False):
    from concourse.bass_utils import run_bass_kernel_spmd

    in_maps, n_chunks = _host_prep(x, hashProj)
    key = (n_chunks, 1)
    if key not in _prog_cache:
        _prog_cache[key] = _build_program(n_chunks)
    nc = _prog_cache[key]

    res = run_bass_kernel_spmd(nc, in_maps, list(range(N_CORES)), trace=trace)
    out_T = np.concatenate([res.results[i]["out"] for i in range(N_CORES)], axis=0)
    out = np.ascontiguousarray(out_T.T, dtype=np.float32)
    return out, res


def kernel(x, hashProj):
    out, _ = _run(x, hashProj)
    return out


# revision 11
# speedup vs baseline: 259.4277x; 1.0021x over previous
"""Trainium2 Bass kernel for Hash1d: out = x @ hashProj.

hashProj is an extremely sparse hash-projection matrix (one +-1 per row), so
out[b, e] = sum_{j: h(j)=e} sign_j * x[b, j] -- a signed segment-sum of x's
columns into E buckets.

Strategy (8 NeuronCores):
  * Host: extract the nonzero entries (col j, bucket e, value v) from
    hashProj, sort them by bucket, and shard *buckets* across the 8 cores
    (core i owns buckets [128*i, 128*(i+1))).  Output shards are disjoint,
    so no collective is needed.
  * Host hands core i a contiguous, transposed slab xs = x.T[cols of core i]
    (features on partitions) padded to a common chunk multiple, plus a tiny
    packed "signed one-hot" matrix w [128 feats x n_chunks*128 local buckets].
  * Device: xs is packed so each DMA group of G chunks is one contiguous-per-
    partition transfer; the PE computes acc[:, bank] += w_k.T @ xs_k for the
    8 PSUM banks (N=512 fp32 moving limit).  All chunks accumulate into one
    full-PSUM [128, 4096] tile, which is copied to SBUF and DMA'd out.
  * Everything is exact fp32 (products are x * +-1), so the result matches
    the fp32 reference to reordering error (~1e-7).

Device traffic per core: ~35 MiB in + 2 MiB out -> ~100 us at ~360 GB/s HBM,
which is at the memory roofline (hashProj's 64 MiB dense zeros never touch
the device).
"""

import numpy as np

BATCH = 4096
INPUT_DIM = 16384
EMB_SIZE = 1024
N_CORES = 8
BPC = EMB_SIZE // N_CORES  # buckets (output partitions) per core = 128
P = 128                    # features per chunk (PE contraction dim)
NFREE = 512                # fp32 moving-operand max free dim = one PSUM bank
NBANK = BATCH // NFREE     # 8 PSUM banks cover the batch
GROUP = 2                  # chunks per xs DMA (4 MiB transfers, best measured)
XBUFS = 4                  # xs group tiles in flight
XS_PAD = 10240             # xs slot padded to 40 KB/partition (SBUF bank spread)
W_ON_ACT = True            # issue w/out DMAs on the ACT HWDGE queue
XS_QUEUES = 1              # 1: all xs DMAs on sync; 2: alternate sync/scalar

_prog_cache = {}


def _chunk_groups(n_chunks):
    """Split chunk indices into DMA groups of size <= GROUP."""
    groups = []
    c = 0
    while c < n_chunks:
        g = min(GROUP, n_chunks - c)
        groups.append((c, g))
        c += g
    return groups


def _build_program(n_chunks, reps=1):
    import concourse.bass as bass
    import concourse.tile as tile
    from concourse import bacc, mybir

    f32 = mybir.dt.float32
    nc = bacc.Bacc("TRN2", target_bir_lowering=False, debug=False)

    # xs packed per group: [128 partitions, g*BATCH] contiguous per partition
    xs_d = nc.dram_tensor("xs", [n_chunks * P * BATCH], f32, kind="ExternalInput")
    # w packed: [128 feat partitions, n_chunks * BPC]
    w_d = nc.dram_tensor("w", [P, n_chunks * BPC], f32, kind="ExternalInput")
    out_d = nc.dram_tensor("out", [BPC, BATCH], f32, kind="ExternalOutput")

    groups = _chunk_groups(n_chunks)

    with tile.TileContext(nc) as tc:
        W_ENG = nc.scalar if W_ON_ACT else nc.sync
        with (
            tc.tile_pool(name="xpool", bufs=XBUFS) as xpool,
            tc.tile_pool(name="wpool", bufs=1) as wpool,
            tc.tile_pool(name="psum", bufs=1, space=bass.MemorySpace.PSUM) as ppool,
            tc.tile_pool(name="opool", bufs=1) as opool,
        ):
            def body(_i):
                wt = wpool.tile([P, n_chunks * BPC], f32)
                W_ENG.dma_start(wt[:], w_d[:])
                acc = ppool.tile([BPC, BATCH], f32)
                for gi, (c0, g) in enumerate(groups):
                    # padded to 40 KB/partition: spreads the 4 rotating slots
                    # across SBUF banks so concurrent DMA writes and PE
                    # moving-operand reads stop colliding (HW: 153us -> 65us)
                    xt = xpool.tile([P, GROUP * BATCH], f32, tag="xs",
                                    padded_shape=[P, XS_PAD])
                    src = xs_d.ap()[c0 * P * BATCH:(c0 + g) * P * BATCH]
                    xeng = nc.scalar if (XS_QUEUES == 2 and gi % 2) else nc.sync
                    xeng.dma_start(
                        xt[:, :g * BATCH],
                        src.rearrange("(p n) -> p n", p=P),
                    )
                    for cl in range(g):
                        k = c0 + cl
                        for n in range(NBANK):
                            nc.tensor.matmul(
                                acc[:, bass.ts(n, NFREE)],
                                wt[:, bass.ts(k, BPC)],
                                xt[:, cl * BATCH + n * NFREE:cl * BATCH + (n + 1) * NFREE],
                                start=(k == 0),
                                stop=(k == n_chunks - 1),
                            )
                out_t = opool.tile([BPC, BATCH], f32)
                for n in range(NBANK):
                    nc.vector.tensor_copy(
                        out_t[:, bass.ts(n, NFREE)], acc[:, bass.ts(n, NFREE)]
                    )
                W_ENG.dma_start(out_d[:], out_t[:])

            if reps == 1:
                body(None)
            else:
                with tc.For_i(0, reps, 1) as i:
                    body(i)

    nc.compile()
    return nc


def _host_prep(x, hashProj):
    """Extract sparse entries, shard by bucket across cores, build per-core inputs."""
    x = np.ascontiguousarray(x, dtype=np.float32)
    hashProj = np.asarray(hashProj, dtype=np.float32)

    # General sparse decomposition: out = sum over nonzeros (j, e, v) of v * x[:, j].
    rows, cols = np.nonzero(hashProj)
    vals = hashProj[rows, cols].astype(np.float32)
    order = np.argsort(cols, kind="stable")
    rows, cols, vals = rows[order], cols[order], vals[order]

    core_of = cols // BPC
    counts = np.bincount(core_of, minlength=N_CORES)
    n_chunks = max(1, -(-int(counts.max()) // P))
    Lp = n_chunks * P

    xT = np.ascontiguousarray(x.T)  # [D, B]: feature-major for partition-dim DMA
    offs = np.zeros(N_CORES + 1, np.int64)
    np.cumsum(counts, out=offs[1:])

    groups = _chunk_groups(n_chunks)

    in_maps = []
    for i in range(N_CORES):
        r = rows[offs[i]:offs[i + 1]]
        c = cols[offs[i]:offs[i + 1]]
        v = vals[offs[i]:offs[i + 1]]
        li = len(r)
        # chunk-major staging: row (k*P + p) = feature p of chunk k
        xs_rows = np.zeros((Lp, BATCH), np.float32)
        if li:
            xs_rows[:li] = xT[r]
        # pack per group: [p, c_local, n] so each group is contiguous per partition
        xs = np.empty(Lp * BATCH, np.float32)
        pos = 0
        for c0, g in groups:
            blk = xs_rows[c0 * P:(c0 + g) * P].reshape(g, P, BATCH)
            xs[pos:pos + g * P * BATCH] = (
                blk.transpose(1, 0, 2).reshape(-1)
            )
            pos += g * P * BATCH
        w = np.zeros((Lp, BPC), np.float32)
        if li:
            w[np.arange(li), c - i * BPC] = v
        # pack w: [p, k*BPC + m]
        w2 = np.ascontiguousarray(
            w.reshape(n_chunks, P, BPC).transpose(1, 0, 2).reshape(P, n_chunks * BPC)
        )
        in_maps.append({"xs": xs, "w": w2})
    return in_maps, n_chunks


def _run(x, hashProj, trace=False):
    from concourse.bass_utils import run_bass_kernel_spmd

    in_maps, n_chunks = _host_prep(x, hashProj)
    key = (n_chunks, 1)
    if key not in _prog_cache:
        _prog_cache[key] = _build_program(n_chunks)
    nc = _prog_cache[key]

    res = run_bass_kernel_spmd(nc, in_maps, list(range(N_CORES)), trace=trace)
    out_T = np.concatenate([res.results[i]["out"] for i in range(N_CORES)], axis=0)
    out = np.ascontiguousarray(out_T.T, dtype=np.float32)
    return out, res


def kernel(x, hashProj):
    out, _ = _run(x, hashProj)
    return out
